# revision 39
# baseline (speedup 1.0000x reference)
"""Trainium2 Bass kernel for nn_AttentionToVec (B=8, N=4096, E=1024, H=16, D=64).

Strategy: data-parallel over batch (1 batch element per NeuronCore) for the
attention part; tensor-parallel over the MLP hidden dim (4096/8=512 per core)
with an AllGather of the per-core sampled vectors and a ReduceScatter of the
partial MLP outputs (which lands exactly each core's own output row).

Algebraic restructuring (host does weight-only folding):
  - att logits = x @ w_att where w_att[e,h] = sum_d W_k[e, h*D+d] * query[h,d]
    (the k-projection bias cancels inside softmax over n).
  - y[h,:] = sum_n softmax_att[n,h] * x[n,:]  (deferred 1/Z normalization)
  - sampled[h,d] = (y[h,:] @ W_v[:, h*D+d]) + b_v[h*D+d]   (sum_n att = 1)

v3 layout plan: x is loaded from HBM twice, in both layouts, as bf16 (host
prepares both; masked rows zeroed). Small transposes (softmax weights, y, S,
h) run as DMA XBAR transposes (2-byte dtype, cheap at these sizes; the big
x transpose is NOT done on-chip because XBAR blocks the issuing engine for
the full transfer). Z comes from the Exp activation's accum_out (minus the
host-counted masked-row count).
"""

import numpy as np

B = 8
N = 4096
E = 1024
H = 16
D = 64
HID = 4096
NCORES = 8
TP_G = 4  # MLP tensor-parallel group size (collective latency scales w/ peers)
HID_C = HID // TP_G
NT = N // 128  # 32 n-tiles
NB = 4         # 1024-wide exp/transpose blocks in phase A2

_CACHE = {}

# Native HW gelu LUT vs a 5-op sigmoid-identity chain (exact same tanh-approx
# formula; CoreSim only implements the chain path).
GELU_NATIVE = True


def _np_bf16():
    import ml_dtypes

    return np.dtype(ml_dtypes.bfloat16)


def _build():
    import concourse.bacc as bacc
    import concourse.mybir as mybir
    from concourse import tile
    import concourse.bass as bass_mod

    f32 = mybir.dt.float32
    bf16 = mybir.dt.bfloat16
    Act = mybir.ActivationFunctionType
    Alu = mybir.AluOpType

    nc = bacc.Bacc(None, target_bir_lowering=False, debug=True, num_devices=NCORES)

    # all big operands are host-prepped in their exact on-chip layouts so every
    # DMA is one contiguous run per partition (descriptor-light triggers)
    xn = nc.dram_tensor("xn", [128, NT, E], bf16, kind="ExternalInput")
    xT = nc.dram_tensor("xT", [E, N], bf16, kind="ExternalInput")
    wattc = nc.dram_tensor("wattc", [128, 8, H], bf16, kind="ExternalInput")
    Wv = nc.dram_tensor("Wv", [128, 8, E], bf16, kind="ExternalInput")
    # packed f32 consts: [bvb | nmask | b1c | b2r8]
    cpack = nc.dram_tensor(
        "cpack", [H, E + 1 + HID_C + E], f32, kind="ExternalInput"
    )
    W1c = nc.dram_tensor("W1c", [128, 8, HID_C], bf16, kind="ExternalInput")
    W2c = nc.dram_tensor("W2c", [128, HID_C // 128, E], bf16, kind="ExternalInput")
    out = nc.dram_tensor("out", [1, E], f32, kind="ExternalOutput")

    with tile.TileContext(nc) as tc:
        with (
            tc.tile_pool(name="consts", bufs=1) as consts,
            tc.tile_pool(name="xsp", bufs=1) as xsp,
            tc.tile_pool(name="xttp", bufs=5) as xttp,
            tc.tile_pool(name="wp", bufs=1) as wp,
            tc.tile_pool(name="work", bufs=1) as work,
            tc.tile_pool(name="dramp", bufs=1, space="DRAM") as dramp,
        ):
            # ---- constant loads (one packed f32 DMA + the bf16 watt) ----
            wattc_s = consts.tile([128, 8, H], bf16)
            nc.sync.dma_start(out=wattc_s[:], in_=wattc[:, :, :])
            cpk = consts.tile([H, E + 1 + HID_C + E], f32)
            nc.sync.dma_start(out=cpk[:], in_=cpack[:, :])
            bvb_s = cpk[:, 0:E]
            nmask_s = cpk[:, E : E + 1]
            b1_s = cpk[0:TP_G, E + 1 : E + 1 + HID_C]
            b28_s = cpk[0:TP_G, E + 1 + HID_C :]

            # S16/hh16 rows 8:16 are read by the XBAR transposes but their
            # transposed columns are never consumed; zero them once up front.
            S16 = work.tile([H, E], bf16, tag="S16")
            nc.vector.memset(S16[:], 0.0)
            hh16 = work.tile([H, HID_C], bf16, tag="hh16")
            nc.vector.memset(hh16[:], 0.0)
            # warm the exp table off the critical path (gelu evicts it later
            # anyway; there is a single table slot so warming gelu is useless)
            warm = work.tile([H, 2], f32, tag="warm")
            nc.scalar.activation(warm[:, 0:1], nmask_s[:], Act.Exp)

            # ---- streamed loads. Time-critical x streams trigger from the
            # sync HWDGE; weights trigger from the scalar HWDGE's own queues.
            xtp_tiles = []
            for c in range(8):
                xt = xttp.tile([128, N], bf16, tag="xT")
                nc.sync.dma_start(out=xt[:], in_=xT[128 * c : 128 * (c + 1), :])
                xtp_tiles.append(xt)

            xs = xsp.tile([128, NT, E], bf16)
            for k in range(8):
                nc.sync.dma_start(
                    out=xs[:, 4 * k : 4 * k + 4, :], in_=xn[:, 4 * k : 4 * k + 4, :]
                )

            wv_s = wp.tile([128, 8, E], bf16, tag="wv")
            nc.scalar.dma_start(out=wv_s[:], in_=Wv[:, :, :])
            w1_s = wp.tile([128, 8, HID_C], bf16, tag="w1")
            nc.scalar.dma_start(out=w1_s[:], in_=W1c[:, :, :])
            w2_s = wp.tile([128, HID_C // 128, E], bf16, tag="w2")
            nc.scalar.dma_start(out=w2_s[:], in_=W2c[:, :, :])

            # ---- Phase A: attT[16, N] = w_att^T @ x^T ----
            psA_cm = tc.tile_pool(name="psA", bufs=1, space="PSUM")
            psA = psA_cm.__enter__()
            attT = psA.tile([H, N], f32)
            for c in range(8):
                xt = xtp_tiles[c]
                for g in range(8):
                    sl = slice(512 * g, 512 * (g + 1))
                    nc.tensor.matmul(
                        attT[:, sl],
                        wattc_s[:, c, :],
                        xt[:, sl],
                        start=(c == 0),
                        stop=(c == 7),
                    )
            # exp per 1024-block (also the PSUM->SBUF move) on scalar, with the
            # matching attn XBAR on the sync HWDGE so the two pipeline; B can
            # start after the first block. accum_out gives Z contributions.
            expm = work.tile([H, N], bf16)
            zparts = work.tile([H, NB], f32)
            attn = work.tile([128, NT, H], bf16)
            TB = NT // NB
            for g in range(NB):
                sl = slice(1024 * g, 1024 * (g + 1))
                nc.scalar.activation(
                    expm[:, sl], attT[:, sl], Act.Exp, accum_out=zparts[:, g : g + 1]
                )
                # attn[p, t, h] = expm[h, t*128+p]
                nc.sync.dma_start(
                    out=attn[:, TB * g : TB * (g + 1), :],
                    in_=expm[:, sl],
                    transpose=True,
                )
            psA_cm.__exit__(None, None, None)

            # Z = sum of parts - (#masked rows); 1/Z
            zsum = work.tile([H, 1], f32)
            nc.vector.tensor_reduce(zsum[:], zparts[:], mybir.AxisListType.X, Alu.add)
            zc = work.tile([H, 1], f32)
            nc.vector.tensor_sub(zc[:], zsum[:], nmask_s[:])
            rz = work.tile([H, 1], f32)
            nc.vector.reciprocal(rz[:], zc[:])

            # ---- Phase B: yhat[16, E] = exp_att^T @ x (accumulate over n) ----
            psB_cm = tc.tile_pool(name="psB", bufs=1, space="PSUM")
            psB = psB_cm.__enter__()
            y_ps = psB.tile([H, E], f32)
            for t in range(NT):
                lhs = attn[:, t, :]
                nc.tensor.matmul(
                    y_ps[:, 0:512], lhs, xs[:, t, 0:512], start=(t == 0), stop=(t == NT - 1)
                )
                nc.tensor.matmul(
                    y_ps[:, 512:1024], lhs, xs[:, t, 512:1024], start=(t == 0), stop=(t == NT - 1)
                )
            y_sb = work.tile([H, E], bf16)
            nc.vector.tensor_copy(y_sb[:], y_ps[:])
            psB_cm.__exit__(None, None, None)

            # yT[p, c, h] = yhat[h, c*128+p]
            yT = work.tile([128, 8, H], bf16)
            nc.scalar.dma_start(out=yT[:], in_=y_sb[:], transpose=True)

            # ---- Phase C: sf = yhat @ Wv; sampled = diag(sf)*rz + bv ----
            psC_cm = tc.tile_pool(name="psC", bufs=1, space="PSUM")
            psC = psC_cm.__enter__()
            sf_ps = psC.tile([H, E], f32)
            for c in range(8):
                for j in range(2):
                    nc.tensor.matmul(
                        sf_ps[:, 512 * j : 512 * (j + 1)],
                        yT[:, c, :],
                        wv_s[:, c, 512 * j : 512 * (j + 1)],
                        start=(c == 0),
                        stop=(c == 7),
                    )
            sf1 = work.tile([H, E], f32)
            nc.vector.tensor_scalar_mul(sf1[:], sf_ps[:], rz[:])
            psC_cm.__exit__(None, None, None)
            sfb = work.tile([H, E], bf16)
            nc.vector.tensor_add(sfb[:], sf1[:], bvb_s[:])

            # sampled[h, d] = sfb[h, h*D + d]: bounce via DRAM with padded rows,
            # then one DRAM->DRAM DMA reads the skewed diagonal view into the
            # contiguous AllGather input (the collective needs a plain AP).
            sf_d = dramp.tile([H, E + D], bf16)
            nc.sync.dma_start(out=sf_d[:, :E], in_=sfb[:])
            sfd_ap = sf_d[:]
            diag_view = bass_mod.AP(
                tensor=sfd_ap.tensor, offset=0, ap=[[E + 2 * D, H], [1, D]]
            )
            s_dram = dramp.tile([1, E], bf16)
            nc.sync.dma_start(
                out=s_dram[:].rearrange("o (h d) -> (o h) d", h=H), in_=diag_view
            )

            # ---- Phase D: AllGather sampled vectors within the TP group ----
            groups = [
                list(range(g * TP_G, (g + 1) * TP_G)) for g in range(NCORES // TP_G)
            ]
            S_all = dramp.tile([TP_G, E], bf16)
            nc.gpsimd.collective_compute(
                "AllGather",
                Alu.bypass,
                replica_groups=groups,
                ins=[s_dram[:].opt()],
                outs=[S_all[:].opt()],
            )

            nc.sync.dma_start(out=S16[0:TP_G, :], in_=S_all[:])
            ST = work.tile([128, 8, H], bf16)
            nc.scalar.dma_start(out=ST[:], in_=S16[:], transpose=True)
            # residual + b2 term only needs S; compute during the MLP matmuls
            sb8 = work.tile([TP_G, E], f32)
            nc.vector.scalar_tensor_tensor(
                sb8[:], S16[0:TP_G, :], 1.0 / TP_G, b28_s[:], Alu.mult, Alu.add
            )

            # ---- Phase E: MLP (tensor-parallel over hidden slice) ----
            psM_cm = tc.tile_pool(name="psM", bufs=1, space="PSUM")
            psM = psM_cm.__enter__()
            h1_ps = psM.tile([TP_G, HID_C], f32, tag="h1")
            for c in range(8):
                for j in range(HID_C // 512):
                    nc.tensor.matmul(
                        h1_ps[:, 512 * j : 512 * (j + 1)],
                        ST[:, c, 0:TP_G],
                        w1_s[:, c, 512 * j : 512 * (j + 1)],
                        start=(c == 0),
                        stop=(c == 7),
                    )
            zb = work.tile([TP_G, HID_C], f32)
            nc.vector.tensor_add(zb[:], h1_ps[:], b1_s[:])
            if GELU_NATIVE:
                nc.scalar.activation(hh16[0:TP_G, :], zb[:], Act.Gelu_apprx_tanh)
            else:
                # gelu_tanh(z) = z * sigmoid(1.5957691...*(z + 0.044715 z^3))
                sq = work.tile([TP_G, HID_C], f32, tag="ga")
                nc.scalar.activation(sq[:], zb[:], Act.Square)
                cb = work.tile([TP_G, HID_C], f32, tag="gb")
                nc.vector.scalar_tensor_tensor(
                    cb[:], sq[:], 0.044715, zb[:], Alu.mult, Alu.mult
                )
                uu = work.tile([TP_G, HID_C], f32, tag="ga")
                nc.vector.tensor_add(uu[:], cb[:], zb[:])
                sg = work.tile([TP_G, HID_C], f32, tag="gb")
                nc.scalar.activation(
                    sg[:], uu[:], Act.Sigmoid, scale=1.5957691216057308
                )
                nc.vector.tensor_mul(hh16[0:TP_G, :], sg[:], zb[:])
            hT = work.tile([128, HID_C // 128, H], bf16)
            nc.scalar.dma_start(out=hT[:], in_=hh16[:], transpose=True)

            p2_ps = psM.tile([TP_G, E], f32, tag="p2")
            for c in range(HID_C // 128):
                for j in range(2):
                    nc.tensor.matmul(
                        p2_ps[:, 512 * j : 512 * (j + 1)],
                        hT[:, c, 0:TP_G],
                        w2_s[:, c, 512 * j : 512 * (j + 1)],
                        start=(c == 0),
                        stop=(c == HID_C // 128 - 1),
                    )
            mlp_s = work.tile([TP_G, E], f32)
            nc.vector.tensor_add(mlp_s[:], p2_ps[:], sb8[:])
            psM_cm.__exit__(None, None, None)
            mlp_d = dramp.tile([TP_G, E], f32)
            nc.sync.dma_start(out=mlp_d[:], in_=mlp_s[:])

            # ---- Phase F: ReduceScatter -> this core's output row ----
            mlp_row = dramp.tile([1, E], f32)
            nc.gpsimd.collective_compute(
                "ReduceScatter",
                Alu.add,
                replica_groups=groups,
                ins=[mlp_d[:].opt()],
                outs=[mlp_row[:].opt()],
            )
            nc.sync.dma_start(out=out[:, :], in_=mlp_row[:])

    return nc


def get_nc():
    if "nc" not in _CACHE:
        nc = _build()
        nc.finalize()
        _CACHE["nc"] = nc
    return _CACHE["nc"]


def build_in_maps(x, mask, W_kv, b_kv, query, W1, b1, W2, b2):
    """Host-side shard prep. Weight-only algebra + layout transforms."""
    bf = _np_bf16()
    x = np.asarray(x, np.float32)
    mask = np.asarray(mask)
    W_kv = np.asarray(W_kv, np.float32)
    b_kv = np.asarray(b_kv, np.float32)
    query = np.asarray(query, np.float32)
    W1 = np.asarray(W1, np.float32)
    b1 = np.asarray(b1, np.float32)
    W2 = np.asarray(W2, np.float32)
    b2 = np.asarray(b2, np.float32)

    W_k = W_kv[:, :E]
    W_v = W_kv[:, E:]
    # fold the per-head query into the k-projection: [E, H]
    w_att = np.einsum("ehd,hd->eh", W_k.reshape(E, H, D), query).astype(np.float32)
    wattc = np.ascontiguousarray(
        w_att.reshape(8, 128, H).transpose(1, 0, 2).astype(bf)
    )  # [p, c, h] with e = c*128 + p
    bv_b = np.ascontiguousarray(
        np.broadcast_to(b_kv[None, E:], (H, E)).astype(np.float32)
    )
    Wv_c = np.ascontiguousarray(
        W_v.astype(bf).reshape(8, 128, E).transpose(1, 0, 2)
    )  # [p, c, e]
    b2r = np.ascontiguousarray(
        np.broadcast_to(b2[None, :] / TP_G, (TP_G, E)).astype(np.float32)
    )

    keep = ~mask[:, :, 0]  # [B, N] True = keep
    nmask_ct = (~keep).sum(axis=1).astype(np.float32)  # [B]

    in_maps = []
    for c in range(NCORES):
        r = c % TP_G  # rank within the MLP tensor-parallel group
        hs = slice(HID_C * r, HID_C * (r + 1))
        xm = np.where(keep[c][:, None], x[c], np.float32(0.0))
        xm_bf = xm.astype(bf)
        cpack = np.zeros((H, E + 1 + HID_C + E), dtype=np.float32)
        cpack[:, 0:E] = bv_b
        cpack[:, E] = nmask_ct[c]
        cpack[0:TP_G, E + 1 : E + 1 + HID_C] = b1[hs][None, :]
        cpack[0:TP_G, E + 1 + HID_C :] = b2r
        in_maps.append(
            {
                "xn": np.ascontiguousarray(
                    xm_bf.reshape(NT, 128, E).transpose(1, 0, 2)
                ),  # [p, t, e] with n = t*128 + p
                "xT": np.ascontiguousarray(xm_bf.T),
                "wattc": wattc,
                "Wv": Wv_c,
                "cpack": cpack,
                "W1c": np.ascontiguousarray(
                    W1[:, hs].astype(bf).reshape(8, 128, HID_C).transpose(1, 0, 2)
                ),
                "W2c": np.ascontiguousarray(
                    W2[hs, :]
                    .astype(bf)
                    .reshape(HID_C // 128, 128, E)
                    .transpose(1, 0, 2)
                ),
            }
        )
    return in_maps


def kernel(**inputs):
    from concourse.bass_utils import run_bass_kernel_spmd

    in_maps = build_in_maps(**inputs)
    nc = get_nc()
    res = run_bass_kernel_spmd(nc, in_maps, list(range(NCORES)), trace=False)
    return np.stack([res.results[c]["out"][0] for c in range(NCORES)]).astype(
        np.float32
    )
